# revision 67
# baseline (speedup 1.0000x reference)
"""KanLinear Trainium2 Bass kernel.

Math (reference):
    base_out  = silu(x) @ scale_base                     # [B,O]
    bases     = exp(-((x[:,:,None]-grid[None])/sigma)^2) # [B,I,G]
    spline    = einsum('big,oig,io->bo', bases, spline_weight, scale_spline)
    out       = base_out + spline

Strategy:
  - Data-parallel shard B=8192 across 8 cores (1024 rows each); params replicated.
  - Host does layout-only prep: x -> x^T slices [I, Bs]; spline_weight [O,I,G]
    -> wT [(g-major k)=G*I, O]; sigma broadcast to [128,1].
  - On device, everything lives in [i/k partitions, b free] layout:
      silu_t = Silu(x^T)                        (ACT, bf16 out)
      u      = Square(x^T * (1/sigma) - grid/sigma)   (ACT, per-partition bias/scale)
      bases  = Exp(-u)                          (ACT, bf16 out)
      w2     = wT_chunk * scale_spline_chunk    (DVE, bf16 out)
      psum[b,o] += silu^T@scale_base + bases^T@w2  (PE, fp32 accum)
  - Single [B, I*G+I] x [I*G+I, O] contraction accumulated in PSUM: for each
    of 4 o-blocks (256 cols), 8 PSUM banks hold the 8 b-blocks, k streams
    over 8 base chunks + 64 spline chunks.
"""

import time

import numpy as np
import orjson

import concourse.bass as bass
import concourse.mybir as mybir
import concourse.tile as tile

F32 = mybir.dt.float32
BF16 = mybir.dt.bfloat16
FP8 = mybir.dt.float8e4

# fp8 weight scaling: spline weights are ~1e-2, below the e4m3 normal range;
# scale up by 2^7 (and scale_base with them) and descale the PSUM drain.
# 2^7 keeps scale_base*WSCALE = 128 under the e4m3 (IEEE) max of 240.
WSCALE = 128.0

N_CORES = 8
B, I, O, G = 8192, 1024, 1024, 8
BS = B // N_CORES  # 1024 batch rows per core
P = 128            # partitions
IB = I // P        # 8 i-blocks
NB = BS // P       # 8 b-blocks
NO = 512           # o-block width (one full PSUM bank tile = [128, 512] f32)
OB = O // NO       # 2 o-blocks
KC = G * IB        # 64 spline k-chunks of 128


MAX_WAIT_SLOTS = 1


def split_sync_waits(bir_json: bytes, max_waits: int = MAX_WAIT_SLOTS) -> bytes:
    """The walrus build in this container rejects instructions with more than
    `max_waits` semaphore wait slots ('Too many sync wait commands').  Move
    excess waits onto NoOps inserted just before the instruction on the same
    engine — the sequencer executes them in order, so the dependency semantics
    are identical."""
    m = orjson.loads(bir_json)
    n_new = 0
    for fn in m["functions"]:
        for blk in fn["blocks"]:
            out_insts = []
            changed = False
            for ins in blk["instructions"]:
                si = ins.get("sync_info")
                waits = (si or {}).get("on_wait") or []
                if len(waits) > max_waits:
                    chunks = [
                        waits[i : i + max_waits]
                        for i in range(0, len(waits), max_waits)
                    ]
                    for chunk in chunks[:-1]:
                        n_new += 1
                        out_insts.append(
                            {
                                "name": f"I-WSPLIT{n_new}",
                                "opcode": "NoOp",
                                "engine": ins["engine"],
                                "ins": [],
                                "outs": [],
                                "sync_info": {"on_wait": chunk, "on_update": []},
                            }
                        )
                    si["on_wait"] = chunks[-1]
                    changed = True
                out_insts.append(ins)
            if changed:
                blk["instructions"] = out_insts
    return orjson.dumps(m)


def install_wait_split_hook():
    """Route every compile through split_sync_waits."""
    from concourse import bass2jax

    if getattr(bass2jax.compile_bir_kernel, "_wait_split", False):
        return
    orig = bass2jax.compile_bir_kernel

    def patched(bir_json, tmpdir, neff_name="file.neff"):
        return orig(split_sync_waits(bir_json), tmpdir, neff_name)

    patched._wait_split = True
    bass2jax.compile_bir_kernel = patched


def build_bass():
    nc = bass.Bass("TRN2", target_bir_lowering=False, debug=False, num_devices=N_CORES)

    # wT is spline_weight pre-folded with scale_spline (constant params),
    # scaled by WSCALE and pre-cast to fp8e4 on the host, laid out as
    # [pair, P, 2, O] so a [128, 2, 512] DoubleRow rhs tile is one DMA;
    # sb is scale_base * WSCALE pre-cast to bf16.
    # consts packs the per-partition RBF-recurrence constants derived from
    # grid/sigma on the host: per ib cols [c1, c2, rsc, cc0..cc6], then inv.
    xT = nc.dram_tensor("xT", [I, BS], F32, kind="ExternalInput").ap()
    wT = nc.dram_tensor("wT", [KC // 2 * P, 2, O], FP8, kind="ExternalInput").ap()
    sb = nc.dram_tensor("sb", [IB // 2 * P, 2, O], FP8, kind="ExternalInput").ap()
    # per ib: [rsc, cc0..cc6, ngs0..ngs7] (16 cols); then [inv, neg1]
    consts = nc.dram_tensor("consts", [P, IB * 16 + 2], F32, kind="ExternalInput").ap()
    out = nc.dram_tensor("out", [BS, O], F32, kind="ExternalOutput").ap()

    AF = mybir.ActivationFunctionType
    ALU = mybir.AluOpType

    with tile.TileContext(nc) as tc:
        with (
            tc.tile_pool(name="const", bufs=1) as const_pool,
            tc.tile_pool(name="xp", bufs=1) as x_pool,
            tc.tile_pool(name="vp", bufs=2) as v_pool,
            tc.tile_pool(name="rp", bufs=2) as r_pool,
            tc.tile_pool(name="silu", bufs=1) as silu_pool,
            tc.tile_pool(name="bases", bufs=1) as bases_pool,
            tc.tile_pool(name="wp", bufs=24) as w_pool,
            tc.tile_pool(name="sbb", bufs=3) as sbb_pool,
            tc.tile_pool(name="psum", bufs=1, space="PSUM") as psum_pool,
            tc.tile_pool(name="op", bufs=3) as out_pool,
        ):
            # ---- phase 0: host-precomputed constants --------------------
            # RBF bases via a per-g multiplicative recurrence (grid columns
            # are uniformly spaced within each row):
            #   b_0     = Exp((2g_0/s^2) x - (x/s)^2 - (g_0/s)^2)
            #   b_{k+1} = b_k * R * cc_k,  R = Exp((2h/s^2) x),
            #   cc_k    = Exp(-h (g_k + g_{k+1}) / s^2)        ([128,1] consts)
            # One DVE STT per grid step; ACT only does Silu/Square/Exp(R)/Exp(b0).
            # consts via the GpSimd queue: its preamble retires ~2us before
            # Sync's, so the first dependent ACT ops can start earlier
            ct = const_pool.tile([P, IB * 16 + 2], F32, tag="consts")
            nc.gpsimd.dma_start(ct[:], consts[:])
            inv_t = ct[:, IB * 16 : IB * 16 + 1]
            neg1_t = ct[:, IB * 16 + 1 : IB * 16 + 2]

            def rsc_(ib):
                return ct[:, ib * 16 : ib * 16 + 1]

            def cc_(ib, k):
                return ct[:, ib * 16 + 1 + k : ib * 16 + 2 + k]

            def ngs_(ib, k):
                return ct[:, ib * 16 + 8 + k : ib * 16 + 9 + k]

            # ---- phase 1: RBF bases (+ silu batches) --------------------
            # bases for one ib live in a 3D [P, G, BS] fp8 tile so a
            # [P, 2, 128] slice is a DoubleRow stationary operand; silu for
            # an ib-pair lives in a [P, 2, BS] fp8 tile for the same reason.
            # Anchors k=0,4 computed directly on ACT: q = Square(x/s - g_k/s),
            # b_k = Exp(-q); k=1..3 / k=5..7 chained on DVE:
            # b_{k+1} = b_k*R*cc_k.  The steady production loop uses only
            # TWO ACT functions (Square, Exp) — the ACT table cache holds two
            # sets, so no ~1.3us table reload per function switch.  Silu (a
            # third function) runs as two batches; base matmuls that consume
            # it sit at the end of each o-pass, so it is off the critical path.
            ANCHORS = (0, 4)
            silu_t = [None] * (IB // 2)
            bases_t = [None] * IB
            x_t = [None] * IB

            def silu_batch(ibs):
                for ib in ibs:
                    if ib % 2 == 0:
                        silu_t[ib // 2] = silu_pool.tile(
                            [P, 2, BS], FP8, tag=f"silu{ib // 2}",
                            name=f"silu{ib // 2}",
                        )
                    nc.scalar.activation(
                        silu_t[ib // 2][:, ib % 2, :], x_t[ib][:], AF.Silu
                    )

            # x0 first (production's critical input), then prefetch the first
            # two weight pairs of pass A ahead of the remaining x DMAs —
            # otherwise the very first matmul waits for a w DMA that sits
            # behind eight x transfers on the Sync queue
            w_pre = []

            def w_prefetch():
                # pairs 0-1 for o0 (pass A) and for o1 (shared by B1 AND B2,
                # which read the same o1 weights) — kills the w-DMA wait at
                # all three pass starts
                for j, (pair, oo) in enumerate(
                    [(0, 0), (1, 0), (0, NO), (1, NO)]
                ):
                    wp = w_pool.tile(
                        [P, 2, NO], FP8, tag=f"wpre{j}", name=f"wpre{j}",
                        bufs=1,
                    )
                    nc.sync.dma_start(
                        wp[:], wT[pair * P : (pair + 1) * P, :, oo : oo + NO]
                    )
                    w_pre.append(wp)

            # warm the ACT Square/Exp tables on a tiny consts slice before x0
            # lands — otherwise the first table load (1.3us each) serializes
            # after the x0 DMA instead of overlapping it
            warm = v_pool.tile([P, 1], F32, tag="warm", name="warm")
            nc.scalar.activation(warm[:], inv_t, AF.Square)
            nc.scalar.activation(warm[:], inv_t, AF.Exp)

            for ib in range(IB):
                xt = x_pool.tile([P, BS], F32, tag=f"x{ib}")
                # x0 via the GpSimd queue too (behind consts), ahead of
                # Sync's preamble tail
                (nc.gpsimd if ib == 0 else nc.sync).dma_start(
                    xt[:], xT[ib * P : (ib + 1) * P, :]
                )
                x_t[ib] = xt
                if ib == 0:
                    w_prefetch()
                bt = bases_pool.tile([P, G, BS], FP8, tag=f"bases{ib}")
                bases_t[ib] = bt
                # ib0 additionally anchors k=1 on ACT so the first spline
                # pair (b0,b1) is ready without waiting on R + the DVE chain
                # + semaphore batching.  The last three ibs anchor k=7 too:
                # with silu deferred, ACT has slack there while DVE (the
                # chain engine) is the binding producer.
                if ib == 0:
                    anchors = (0, 1, 4)
                elif ib >= IB - 3:
                    anchors = (0, 4, 7)
                else:
                    anchors = ANCHORS

                def anchor(k):
                    qt = v_pool.tile([P, BS], F32, tag=f"q{k}", name=f"q{k}")
                    nc.scalar.activation(
                        qt[:], xt[:], AF.Square, scale=inv_t, bias=ngs_(ib, k)
                    )
                    nc.scalar.activation(bt[:, k, :], qt[:], AF.Exp, scale=neg1_t)

                rt = r_pool.tile([P, BS], F32)
                if ib == 0:
                    anchor(0)
                    anchor(1)
                    nc.scalar.activation(rt[:], xt[:], AF.Exp, scale=rsc_(ib))
                    anchor(4)
                else:
                    anchor(0)
                    nc.scalar.activation(rt[:], xt[:], AF.Exp, scale=rsc_(ib))
                    anchor(4)
                    if 7 in anchors:
                        anchor(7)
                for k in range(G):
                    if k in anchors:
                        continue
                    nc.vector.scalar_tensor_tensor(
                        bt[:, k, :], rt[:], cc_(ib, k - 1), bt[:, k - 1, :],
                        ALU.mult, ALU.mult,
                    )
            # silu emitted AFTER all production ACT ops: it is only needed by
            # the base matmuls late in pass A, and putting it mid-production
            # delayed the later ibs' anchors (stalling their DVE chains and
            # the PE pairs pacing on them)
            silu_batch(range(IB))

            # ---- phase 2: matmuls ---------------------------------------
            # k-chunk order kc = ib*G + g matches phase-1 production order,
            # so the PE never waits long for a base tile.
            # Pass A: (all b, o 0:512) — 8 banks, production-paced.
            # Passes B1/B2: (b 0:4, o 512:1024) then (b 4:8, o 512:1024) —
            # 4 banks each, so B1's drains + out-DMA (1MB) overlap B2's
            # matmuls instead of all 2MB of output draining after the very
            # last matmul.
            def mm_pass(b_lo, b_hi, o0, pre_base, pname):
                nbp = b_hi - b_lo
                psums = [
                    psum_pool.tile(
                        [P, NO], F32, tag=f"ps{b}", name=f"ps_{pname}_{b}"
                    )
                    for b in range(b_lo, b_hi)
                ]

                def base_mm(ibp):
                    sb_b = sbb_pool.tile([P, 2, NO], FP8)
                    nc.sync.dma_start(
                        sb_b[:], sb[ibp * P : (ibp + 1) * P, :, o0 : o0 + NO]
                    )
                    for b in range(b_lo, b_hi):
                        nc.tensor.matmul(
                            psums[b - b_lo][:],
                            silu_t[ibp][:, :, b * P : (b + 1) * P],
                            sb_b[:],
                            start=False,
                            stop=(ibp == IB // 2 - 1),
                            perf_mode=mybir.MatmulPerfMode.DoubleRow,
                            skip_group_check=True,
                        )

                # base MMs late in the pass: silu is produced after the
                # production window, ready by the time the PE (production-
                # paced in pass A) reaches pair 26
                base_at = {26: 0, 28: 1, 30: 2}
                for pair in range(KC // 2):
                    ib, gp = pair // (G // 2), pair % (G // 2)
                    if pair < 2:
                        w_t = w_pre[pre_base + pair]
                    else:
                        w_t = w_pool.tile([P, 2, NO], FP8)
                        # in the B passes ACT is idle — alternate w DMAs onto
                        # its queue to halve Sync-queue pressure (in pass A
                        # ACT is production-busy, keep everything on Sync)
                        weng = nc.scalar if (pre_base and pair % 2) else nc.sync
                        weng.dma_start(
                            w_t[:], wT[pair * P : (pair + 1) * P, :, o0 : o0 + NO]
                        )
                    for b in range(b_lo, b_hi):
                        nc.tensor.matmul(
                            psums[b - b_lo][:],
                            bases_t[ib][:, 2 * gp : 2 * gp + 2, b * P : (b + 1) * P],
                            w_t[:],
                            start=(pair == 0),
                            stop=False,
                            perf_mode=mybir.MatmulPerfMode.DoubleRow,
                            skip_group_check=True,
                        )
                    if pair in base_at:
                        base_mm(base_at[pair])
                base_mm(IB // 2 - 1)

                # drain (descale by 1/WSCALE) PSUM -> SBUF -> DRAM.  Each
                # bank is split into two half-width ops running on DVE and
                # ACT concurrently; out-DMAs issue from three different
                # engine queues so the final writes overlap.
                H = NO // 2
                for b in range(b_lo, b_hi):
                    o_t = out_pool.tile([P, NO], F32)
                    nc.vector.tensor_scalar_mul(
                        o_t[:, 0:H], psums[b - b_lo][:, 0:H], 1.0 / WSCALE
                    )
                    nc.scalar.activation(
                        o_t[:, H:NO], psums[b - b_lo][:, H:NO],
                        AF.Copy, scale=1.0 / WSCALE,
                    )
                    dma_eng = (nc.sync, nc.scalar, nc.gpsimd)[b % 3]
                    dma_eng.dma_start(
                        out[b * P : (b + 1) * P, o0 : o0 + NO], o_t[:]
                    )

            # HAM pre-warm: the PE clock gate defaults to half rate and needs
            # ~3.4us of sustained activity to open.  A burst of dummy f32
            # matmuls on the consts tile spans the otherwise-idle lead-in
            # window so the real stream starts at full clock.  The dummies
            # borrow pass A's bank-7 slot via the tag ring (results are
            # never read; pass A's start=True clears the bank).
            warm_ps = psum_pool.tile([P, NO], F32, tag="ps7", name="warm_ps")
            for _ in range(36):
                nc.tensor.matmul(
                    warm_ps[:, 0:P], ct[:, 0:P], ct[:, 0:P],
                    start=True, stop=True, skip_group_check=True,
                )

            # asymmetric 5/3 final split: the last pass's end-loaded drain
            # shrinks to 3 banks (768KB) while 3 banks still consume w pairs
            # slowly enough for the DMA queues to keep ahead
            mm_pass(0, NB, 0, 0, "a")
            mm_pass(0, 5, NO, 2, "b1")
            mm_pass(5, NB, NO, 2, "b2")

    return nc


# ---------------------------------------------------------------------------
# host-side runner: build + compile once, then execute on 8 cores via PJRT
# ---------------------------------------------------------------------------
_STATE = {}


def _get_runner():
    if "run" in _STATE:
        return _STATE["run"]

    import jax
    from jax.sharding import Mesh, PartitionSpec
    from jax.experimental.shard_map import shard_map
    from concourse import bass2jax
    from concourse import mybir as _mb

    nc = build_bass()
    install_wait_split_hook()
    bass2jax.install_neuronx_cc_hook()

    partition_name = nc.partition_id_tensor.name if nc.partition_id_tensor else None
    in_names, out_names, out_avals, zero_shapes = [], [], [], []
    for alloc in nc.m.functions[0].allocations:
        if not isinstance(alloc, _mb.MemoryLocationSet):
            continue
        name = alloc.memorylocations[0].name
        if alloc.kind == "ExternalInput":
            if name != partition_name:
                in_names.append(name)
        elif alloc.kind == "ExternalOutput":
            out_names.append(name)
            shape = tuple(alloc.tensor_shape)
            dtype = _mb.dt.np(alloc.dtype)
            out_avals.append(jax.core.ShapedArray(shape, dtype))
            zero_shapes.append((shape, dtype))
    n_params = len(in_names)
    n_outs = len(out_avals)
    all_in_names = in_names + out_names
    if partition_name is not None:
        all_in_names = all_in_names + [partition_name]

    donate = tuple(range(n_params, n_params + n_outs))

    def _body(*args):
        operands = list(args)
        if partition_name is not None:
            operands.append(bass2jax.partition_id_tensor())
        outs = bass2jax._bass_exec_p.bind(
            *operands,
            out_avals=tuple(out_avals),
            in_names=tuple(all_in_names),
            out_names=tuple(out_names),
            lowering_input_output_aliases=(),
            sim_require_finite=True,
            sim_require_nnan=True,
            nc=nc,
        )
        return tuple(outs)

    devices = jax.devices()[:N_CORES]
    mesh = Mesh(np.asarray(devices), ("core",))
    specs = (PartitionSpec("core"),) * (n_params + n_outs)
    sharded = jax.jit(
        shard_map(
            _body,
            mesh=mesh,
            in_specs=specs,
            out_specs=(PartitionSpec("core"),) * n_outs,
            check_rep=False,
        ),
        donate_argnums=donate,
        keep_unused=True,
    )

    def run(in_maps):
        concat_in = [
            np.concatenate([np.asarray(in_maps[c][nm]) for c in range(N_CORES)], axis=0)
            for nm in in_names
        ]
        concat_zeros = [
            np.zeros((N_CORES * s[0], *s[1:]), d) for (s, d) in zero_shapes
        ]
        out_arrs = sharded(*concat_in, *concat_zeros)
        return [
            {
                nm: np.asarray(out_arrs[i]).reshape(N_CORES, *out_avals[i].shape)[c]
                for i, nm in enumerate(out_names)
            }
            for c in range(N_CORES)
        ]

    from jax.sharding import NamedSharding

    sh = NamedSharding(mesh, PartitionSpec("core"))

    def prep(in_maps):
        concat_in = [
            np.concatenate([np.asarray(in_maps[c][nm]) for c in range(N_CORES)], axis=0)
            for nm in in_names
        ]
        dev_in = [jax.device_put(a, sh) for a in concat_in]
        jax.block_until_ready(dev_in)
        return dev_in

    def exec_once(dev_in):
        zeros = [
            jax.device_put(np.zeros((N_CORES * s[0], *s[1:]), d), sh)
            for (s, d) in zero_shapes
        ]
        jax.block_until_ready(zeros)
        t0 = time.perf_counter()
        outs = sharded(*dev_in, *zeros)
        jax.block_until_ready(outs)
        return time.perf_counter() - t0

    def timed(in_maps, iters=20):
        """Steady-state timing: inputs device-resident; only fresh donated
        zero output buffers are re-staged (outside the timed region)."""
        dev_in = prep(in_maps)
        times = [exec_once(dev_in) for _ in range(iters)]
        return min(times) * 1e9, times

    _STATE["run"] = run
    _STATE["timed"] = timed
    _STATE["prep"] = prep
    _STATE["exec"] = exec_once
    _STATE["nc"] = nc
    return run


def _make_in_maps(x, scale_base, spline_weight, scale_spline, grid, sigma):
    import ml_dtypes

    bf16 = ml_dtypes.bfloat16
    x = np.asarray(x, np.float32)
    grid = np.asarray(grid, np.float32)
    sig = np.float32(np.asarray(sigma))

    # Per-partition RBF-recurrence constants (derived from grid/sigma):
    # per ib: [rsc, cc0..cc6, ngs0..ngs7]; final cols = [1/sigma, -1].
    gs = grid / sig                                    # [I, G]
    hs = gs[:, 1] - gs[:, 0]                           # h/sigma per row
    consts = np.zeros((P, IB * 16 + 2), np.float32)
    for ib in range(IB):
        rows = slice(ib * P, (ib + 1) * P)
        consts[:, ib * 16 + 0] = 2.0 * hs[rows] / sig              # rsc
        consts[:, ib * 16 + 1 : ib * 16 + 8] = np.exp(
            -hs[rows, None] * (gs[rows, :-1] + gs[rows, 1:])
        )                                                          # cc_k
        consts[:, ib * 16 + 8 : ib * 16 + 16] = -gs[rows, :]       # -g_k/s
    consts[:, IB * 16] = 1.0 / sig
    consts[:, IB * 16 + 1] = -1.0

    xT = np.ascontiguousarray(x.T)  # [I, B]
    # Constant folding: w2[o,i,g] = spline_weight[o,i,g] * scale_spline[i,o]
    # (both fixed params), scaled by WSCALE and cast fp8e4.  Pair layout for
    # DoubleRow: [IB, G/2, P, 2, O] so tile [128, 2, 512] is one DMA.
    w2 = np.asarray(spline_weight, np.float32).transpose(1, 2, 0)  # [I, G, O]
    w2 = w2 * np.asarray(scale_spline, np.float32)[:, None, :] * np.float32(WSCALE)
    wT = np.ascontiguousarray(
        w2.reshape(I // P, P, G, O)
        .transpose(0, 2, 1, 3)       # [IB, G, P, O]
        .reshape(I // P, G // 2, 2, P, O)
        .transpose(0, 1, 3, 2, 4)    # [IB, G/2, P, 2, O]
        .reshape(KC // 2 * P, 2, O)
        .astype(ml_dtypes.float8_e4m3)
    )
    # scale_base * WSCALE in fp8, ib-pair layout [IB/2, P, 2, O] for DoubleRow
    sb8 = np.ascontiguousarray(
        (np.asarray(scale_base, np.float32) * np.float32(WSCALE))
        .reshape(IB // 2, 2, P, O)
        .transpose(0, 2, 1, 3)       # [IB/2, P, 2, O]
        .reshape(IB // 2 * P, 2, O)
        .astype(ml_dtypes.float8_e4m3)
    )

    in_maps = []
    for c in range(N_CORES):
        in_maps.append(
            {
                "xT": np.ascontiguousarray(xT[:, c * BS : (c + 1) * BS]),
                "wT": wT,
                "sb": sb8,
                "consts": consts,
            }
        )
    return in_maps


def kernel(x, scale_base, spline_weight, scale_spline, grid, sigma):
    run = _get_runner()
    in_maps = _make_in_maps(x, scale_base, spline_weight, scale_spline, grid, sigma)
    results = run(in_maps)
    return np.concatenate([results[c]["out"] for c in range(N_CORES)], axis=0)


def timed_run(inputs, iters=20):
    """Min wall-clock (ns) of a steady-state device-resident invocation."""
    _get_runner()
    in_maps = _make_in_maps(**inputs)
    best_ns, times = _STATE["timed"](in_maps, iters)
    ms = ", ".join(f"{t * 1e3:.2f}" for t in sorted(times)[:5])
    print(f"  fastest runs (ms): {ms}")
    return best_ns


def profile_run(inputs, outdir, cores=(0,)):
    """Capture an NTFF profile of one execution via the axon sidechannel;
    returns (max exec_time_ns over profiled cores, perfetto_trace_path).
    Work is SPMD-symmetric across the 8 cores, so core 0 is representative
    (verified: core 0 vs core 7 agree within noise)."""
    import glob
    import os

    from trn_agent_boot.trn_boot import _ntff_profile_via_ctypes

    import gauge.profiler
    from concourse.bass_utils import FishPath

    _get_runner()
    in_maps = _make_in_maps(**inputs)
    dev_in = _STATE["prep"](in_maps)
    _STATE["exec"](dev_in)  # warmup

    os.makedirs(outdir, exist_ok=True)
    hook = _ntff_profile_via_ctypes("/opt/axon/libaxon_pjrt.so")
    with hook(outdir, list(cores)):
        _STATE["exec"](dev_in)

    ntffs = glob.glob(os.path.join(outdir, "*_body*.ntff")) or glob.glob(
        os.path.join(outdir, "*.ntff")
    )
    if not ntffs:
        raise RuntimeError(f"no NTFF files written to {outdir}")
    profile = gauge.profiler.Profile(
        profile_path=FishPath(outdir),
        kernel_dev_mode=True,
        profile_on_exit=False,
        bass_kernel=_STATE["nc"].m,
        offline_processing=True,
        fname="*_body*",
    )
    results = profile.to_perfetto(model_index=tuple(cores))
    exec_ns = max(r.exec_time_ns for r in results)
    return exec_ns, results[0].trace_path

